# revision 8
# baseline (speedup 1.0000x reference)
"""MultiHeadAttention TRN2 Bass kernel, v2 (balanced ACT/DVE softmax).

Problem: B=16, L=1024, F=512, H=8 heads, D=64.
  q = Q@Wq+bq; k = K@Wk+bk(=0-folded); v = V@Wv+bv
  S = q k^T / sqrt(D); P = softmax(S); ctx = P v
  out = tanh(concat([ctx, Q]) @ Wo + bo)

Sharding: data-parallel over batch, 2 batches per core, 8 cores.

v2 design (cost-model driven; bottleneck = PSUM-evacuation on ACT+DVE):
  - All inputs host-pre-TRANSPOSED so every load is a plain DMA (no XBAR),
    roughly halving DMA-device occupancy vs transposed loads.
  - fp8e4m3 DoubleRow matmuls for proj / scores / ctx / out-proj(ctx half);
    fp16 skip path (precision-critical: concat-Q @ WoB).
  - Scores pre-scaled by A16=2^10/ln2 (folded into q/k evac scales).
    All P is stored e5m2; exp is split between ACT and DVE per kt tile
    (per-head ratios tuned by a greedy search over the cost model):
      * ACT tiles: exact exp activation -> e5m2.
      * DVE tiles: Schraudolph exp via ONE tensor_scalar:
        uint8(round(S~/256 + C_ADD)) bitcast as e5m2 IS exp(S-SHIFT);
        uint8 conversion saturates at 0 => tiny P for very negative S,
        and bits<=127 for S<=13, so no overflow either.
    ctx matmuls consume kt pairs as e5m2 DoubleRow (0.5 cyc/col).
  - ctx layout B: psum [128q, 8qt, D+1] with an appended ones column on v
    giving the softmax denominator Z for free; normalize = one reciprocal
    + one scalar_tensor_tensor per head (DVE), writing fp8 ctx_n.
  - ctx transpose for the output projection is done by the DMA XBAR
    (SBUF->SBUF u16-pair transposes), costing zero ACT/DVE/PE time.
  - out-proj psum [128f, L] per fo-chunk: 2 DR (ctx) + 4 fp16 (skip)
    matmuls, tanh evac on ACT with per-partition bo bias; outputs stored
    transposed (host undoes for free).
"""

import numpy as np
import ml_dtypes

import concourse.bass as bass
import concourse.tile as tile
from concourse import bacc, mybir
from concourse import bass_utils

B, L, F, H, D = 16, 1024, 512, 8, 64
NCORES = 8
BPC = B // NCORES
F32 = mybir.dt.float32
F16 = mybir.dt.float16
FP8 = mybir.dt.float8e4
FP8E5 = mybir.dt.float8e5
U16 = mybir.dt.uint16
U8 = mybir.dt.uint8

A16 = 1024.0 / np.log(2.0)        # 1477.32; S~ = A16 * S_true in psum
ALPHA = float(np.sqrt(A16 / 8.0))  # q/k evac scale so S~ = A16*S
SHIFT = 3.0                        # P = exp(S - SHIFT); S<=8.2 so P<=180<240
C_ADD = 60.0 - 4.0 * SHIFT / np.log(2.0) - 0.3  # e5m2 Schraudolph bias
W_SCALE = 32.0
CTX_SCALE = 16.0
WO_SCALE = 16.0

# exp engine per (batch, head, kt): A=ACT exact-exp, D=DVE schraudolph;
# all P is e5m2 so the engine choice is free per tile; alternate for
# pipelining. Per-head ACT counts found by greedy search over the
# TimelineSim makespan (correctness is invariant to this choice).
_PATS = {
    3: ("A", "D", "D", "A", "D", "D", "A", "D"),
    4: ("A", "D", "A", "D", "A", "D", "D", "A"),
    5: ("A", "D", "A", "D", "A", "D", "A", "A"),
    6: ("A", "A", "D", "A", "D", "A", "A", "A"),
}
_ACNT = {(0, 0): 4, (0, 1): 4, (0, 2): 5, (0, 3): 5, (0, 4): 5, (0, 5): 4,
         (0, 6): 5, (0, 7): 4, (1, 0): 5, (1, 1): 5, (1, 2): 6, (1, 3): 4,
         (1, 4): 5, (1, 5): 4, (1, 6): 5, (1, 7): 4}
TILE_ENG = {}
for _b in range(BPC):
    for _h in range(H):
        for _kt in range(8):
            TILE_ENG[(_b, _h, _kt)] = _PATS[_ACNT[(_b, _h)]][_kt]
# per-tile overrides from a randomized local search over the simulated
# makespan (correctness is invariant to the engine assignment)
TILE_ENG.update({
    (0, 0, 2): "D", (0, 0, 5): "A", (1, 1, 0): "D", (1, 1, 7): "D",
    (1, 2, 6): "D", (1, 3, 5): "A", (1, 3, 7): "D", (1, 4, 7): "D",
    (1, 5, 3): "A", (1, 7, 5): "A",
})

# evac engines per chunk parity / v-pair parity, filler counts (searched)
EVACQ = ("D", "A", "D", "A")   # per ch
EVACK = ("D", "A", "D", "A")
EVACV = ("D", "A", "D", "A")   # per kt-pair j
WARM_N = 8
TAIL_N = 6

_CACHE = {}
E4M3 = ml_dtypes.float8_e4m3


def _build_program():
    nc = bacc.Bacc("TRN2", target_bir_lowering=False)

    dIn = nc.dram_tensor("qkv", (BPC, F, 3, L), U8, kind="ExternalInput")
    dQ16 = nc.dram_tensor("q16", (BPC, F, L), F16, kind="ExternalInput")
    dWall = nc.dram_tensor("wall", (128, 12320), U8, kind="ExternalInput")
    dOut = nc.dram_tensor("outT", (BPC, F, L), F16, kind="ExternalOutput")

    with tile.TileContext(nc) as tc:
        _kernel(tc, dIn, dQ16, dWall, dOut)

    nc.compile()
    return nc


def _kernel(tc, dIn, dQ16, dWall, dOut):
    nc = tc.nc
    Exp = mybir.ActivationFunctionType.Exp
    Tanh = mybir.ActivationFunctionType.Tanh
    Ident = mybir.ActivationFunctionType.Identity
    Copy = mybir.ActivationFunctionType.Copy
    Mult = mybir.AluOpType.mult
    Add = mybir.AluOpType.add
    DR = mybir.MatmulPerfMode.DoubleRow

    from contextlib import ExitStack
    ctx = ExitStack()
    consts = ctx.enter_context(tc.tile_pool(name="consts", bufs=1))
    p_in = ctx.enter_context(tc.tile_pool(name="p_in", bufs=2))
    p_qk = ctx.enter_context(tc.tile_pool(name="p_qk", bufs=2))
    p_va = ctx.enter_context(tc.tile_pool(name="p_va", bufs=2))
    p_exp = ctx.enter_context(tc.tile_pool(name="p_exp", bufs=3))
    p_cn = ctx.enter_context(tc.tile_pool(name="p_cn", bufs=2))
    p_out = ctx.enter_context(tc.tile_pool(name="p_out", bufs=2))
    p_rc = ctx.enter_context(tc.tile_pool(name="p_rc", bufs=2))
    psS = ctx.enter_context(tc.tile_pool(name="psS", bufs=3, space="PSUM"))
    psC = ctx.enter_context(tc.tile_pool(name="psC", bufs=1, space="PSUM"))

    # ------------------------- loads -----------------------------------
    INs, QT16s = [], []
    for b in range(BPC):
        INs.append(p_in.tile([128, 4, 3, L], U8, tag="inall", name=f"in_{b}"))
        QT16s.append(p_in.tile([128, 4, L], F16, tag="qt16", name=f"qt16_{b}"))

    wall = consts.tile([128, 12320], U8, tag="wall")

    def load_t(b, t):
        nc.sync.dma_start(
            out=INs[b][:, :, t, :],
            in_=dIn[b, :, t, :].rearrange("(c p) l -> p c l", c=4))

    def load_q16(b):
        nc.sync.dma_start(
            out=QT16s[b],
            in_=dQ16[b].rearrange("(c p) l -> p c l", c=4))

    def load_qk(b):
        nc.sync.dma_start(
            out=INs[b][:, :, 0:2, :],
            in_=dIn[b, :, 0:2, :].rearrange("(c p) t l -> p c t l", c=4))

    # biases+Wq+Wk first, then weights in consumption order; q16 last
    nc.sync.dma_start(out=wall[:, 0:4128], in_=dWall[:, 0:4128])
    load_t(0, 0)
    load_t(0, 1)
    nc.sync.dma_start(out=wall[:, 4128:8224], in_=dWall[:, 4128:8224])
    load_t(0, 2)
    nc.sync.dma_start(out=wall[:, 8224:12320], in_=dWall[:, 8224:12320])
    load_qk(1)
    load_t(1, 2)
    load_q16(0)
    load_q16(1)

    bq_sb = wall[:, 0:16].bitcast(F32)
    bo_sb = wall[:, 16:32].bitcast(F32)
    Wq_sb = wall[:, 32:2080].bitcast(FP8).rearrange(
        "p (u i f) -> p u i f", u=2, i=2)
    Wk_sb = wall[:, 2080:4128].bitcast(FP8).rearrange(
        "p (u i f) -> p u i f", u=2, i=2)
    Wv_sb = wall[:, 4128:6176].bitcast(FP8).rearrange(
        "p (u i f) -> p u i f", u=2, i=2)
    WoT_sb = wall[:, 6176:8224].bitcast(FP8).rearrange(
        "p (c i f) -> p c i f", c=2, i=2)
    WoB_sb = wall[:, 8224:12320].bitcast(F16).rearrange(
        "p (c f) -> p c f", c=4)
    nshift_sb = consts.tile([128, 1], F32, tag="nshift")
    nc.gpsimd.memset(nshift_sb, -SHIFT)
    warm = consts.tile([128, 512], F16, tag="warm")
    nc.gpsimd.memset(warm, 0.5)
    # dummy exp forces the ACT table load at t~0 instead of lazily at
    # the first real activation
    actprime = consts.tile([128, 1], F32, tag="actprime")
    nc.scalar.activation(out=actprime, in_=nshift_sb, func=Exp,
                         scale=1.0, bias=nshift_sb[:, 0:1])

    st = {}

    # ------------------------- projections -----------------------------
    def proj_qk(b, chs):
        ia = INs[b].bitcast(FP8)
        Q8, K8 = ia[:, :, 0, :], ia[:, :, 1, :]
        if chs[0] == 0:
            st[b]["qs8"] = p_qk.tile([128, 2, 2, L], FP8, tag="qs8",
                                     name=f"qs8_{b}")
            st[b]["ks8"] = p_qk.tile([128, 2, 2, L], FP8, tag="ks8",
                                     name=f"ks8_{b}")
        qs8, ks8 = st[b]["qs8"], st[b]["ks8"]
        for ch in chs:
            ps = psS.tile([128, 1024], F32, tag="S", name=f"qp{b}{ch}")
            for nh in range(2):
                sl = slice(nh * 512, (nh + 1) * 512)
                for u in range(2):
                    nc.tensor.matmul(
                        ps[:, sl], Wq_sb[:, u, :, ch * 128:(ch + 1) * 128],
                        Q8[:, 2 * u:2 * u + 2, sl],
                        start=(u == 0), stop=(u == 1), perf_mode=DR)
            if EVACQ[ch] == "A":
                nc.scalar.activation(
                    out=qs8[:, ch // 2, ch % 2, :], in_=ps, func=Ident,
                    bias=bq_sb[:, ch:ch + 1], scale=ALPHA / W_SCALE)
            else:
                nc.vector.tensor_scalar(
                    out=qs8[:, ch // 2, ch % 2, :], in0=ps,
                    scalar1=ALPHA / W_SCALE, scalar2=bq_sb[:, ch:ch + 1],
                    op0=Mult, op1=Add)
            ps = psS.tile([128, 1024], F32, tag="S", name=f"kp{b}{ch}")
            for nh in range(2):
                sl = slice(nh * 512, (nh + 1) * 512)
                for u in range(2):
                    nc.tensor.matmul(
                        ps[:, sl], Wk_sb[:, u, :, ch * 128:(ch + 1) * 128],
                        K8[:, 2 * u:2 * u + 2, sl],
                        start=(u == 0), stop=(u == 1), perf_mode=DR)
            if EVACK[ch] == "D":
                nc.vector.tensor_scalar(
                    out=ks8[:, ch // 2, ch % 2, :], in0=ps,
                    scalar1=ALPHA / W_SCALE, scalar2=None, op0=Mult)
            else:
                nc.scalar.activation(
                    out=ks8[:, ch // 2, ch % 2, :], in_=ps, func=Copy,
                    scale=ALPHA / W_SCALE)

    def proj_v(b):
        ia = INs[b].bitcast(FP8)
        V8 = ia[:, :, 2, :]
        vaug8 = p_va.tile([128, 8, H, D + 1], FP8, tag="v8", name=f"v8_{b}")
        st[b]["vaug8"] = vaug8
        nc.gpsimd.memset(vaug8[:, :, :, D:D + 1], 1.0)
        for j in range(4):
            ps = psS.tile([128, 1024], F32, tag="S", name=f"vp{b}{j}")
            psv = ps.rearrange("p (s n) -> p s n", s=2)
            for s in range(2):
                kt = 2 * j + s
                for u in range(2):
                    nc.tensor.matmul(
                        psv[:, s, :],
                        V8[:, 2 * u:2 * u + 2, kt * 128:(kt + 1) * 128],
                        Wv_sb[:, u, :, :],
                        start=(u == 0), stop=(u == 1), perf_mode=DR)
            if EVACV[j] == "D":
                nc.vector.tensor_scalar(
                    out=vaug8[:, 2 * j:2 * j + 2, :, 0:D],
                    in0=psv.rearrange("p s (h d) -> p s h d", h=H),
                    scalar1=1.0 / W_SCALE, scalar2=None, op0=Mult)
            else:
                nc.scalar.activation(
                    out=vaug8[:, 2 * j:2 * j + 2, :, 0:D],
                    in_=psv.rearrange("p s (h d) -> p s h d", h=H),
                    func=Copy, scale=1.0 / W_SCALE)

    # ------------------------- attention -------------------------------
    def scores(b, h, splice=()):
        # `splice`: PE-matmul closures interleaved between kt tiles so
        # bulky side work never blocks the scores pipeline refill
        splice = list(splice)
        qs8, ks8 = st[b]["qs8"], st[b]["ks8"]
        pb, gg = 32 * (h % 4), h // 4
        e8 = p_exp.tile([128, 8, L], FP8, tag="e8", name=f"e8_{b}_{h}")
        e8u = e8.bitcast(U8)
        st[(b, h)] = e8
        for kt in range(8):
            for _ in range(2):
                if splice:
                    splice.pop(0)()
            ps = psS.tile([128, 1024], F32, tag="S", name=f"s{b}{h}{kt}")
            for nh in range(2):
                sl = slice(nh * 512, (nh + 1) * 512)
                nc.tensor.matmul(
                    ps[:, sl], ks8[pb:pb + 32, gg, :, kt * 128:(kt + 1) * 128],
                    qs8[pb:pb + 32, gg, :, sl],
                    start=True, stop=True, perf_mode=DR,
                    tile_position=(pb, 0))
            if TILE_ENG[(b, h, kt)] == "A":
                nc.scalar.activation(
                    out=e8.bitcast(FP8E5)[:, kt, :], in_=ps, func=Exp,
                    scale=1.0 / A16, bias=nshift_sb[:, 0:1])
            else:
                nc.vector.tensor_scalar(
                    out=e8u[:, kt, :], in0=ps, scalar1=1.0 / 256.0,
                    scalar2=C_ADD, op0=Mult, op1=Add)

    def head_ctx(b, h):
        e8 = st[(b, h)]
        vaug8, ctx_n = st[b]["vaug8"], st[b]["ctx_n"]
        e85 = e8.bitcast(FP8E5)
        # single 2-bank cps tile per head; qt stride padded to 128 floats
        # (512B) so every accumulation group stays within one psum bank
        cps = psC.tile([128, 8, 128], F32, tag="C", name=f"c{b}{h}")
        for qt in range(8):
            for j in range(4):
                nc.tensor.matmul(
                    cps[:, qt, 0:D + 1],
                    e85[:, 2 * j:2 * j + 2, qt * 128:(qt + 1) * 128],
                    vaug8[:, 2 * j:2 * j + 2, h, :],
                    start=(j == 0), stop=(j == 3),
                    perf_mode=DR)
        rc = p_rc.tile([128, 8], F32, tag="rc", name=f"rc{b}{h}")
        nc.vector.reciprocal(out=rc, in_=cps[:, :, D])
        rcb = rc.unsqueeze(2).broadcast_to([128, 8, D])
        nc.vector.scalar_tensor_tensor(
            out=ctx_n[:, :, h, :],
            in0=cps[:, :, 0:D], scalar=CTX_SCALE, in1=rcb,
            op0=Mult, op1=Mult)

    def ctx_transpose(b):
        # two XBAR transposes (one per q-half) into SEPARATE tiles so the
        # tile framework doesn't serialize the writers:
        # out[p, g=2qt+cp, q] = cn16[q, 128*g + p]  (probe-verified)
        cn16 = st[b]["ctx_n"].bitcast(U16).rearrange(
            "p a h d -> p (a h d)")            # [128, 2048]
        for nh in range(2):
            ctxT = p_cn.tile([128, 8, 128], U16, tag=f"ctxT{nh}",
                             name=f"ctxT_{b}_{nh}")
            st[b, "ctxT", nh] = ctxT
            nc.sync.dma_start_transpose(
                out=ctxT, in_=cn16[:, nh * 1024:(nh + 1) * 1024])

    def outproj_skip(b, fo):
        # skip-path (Q16) matmuls FIRST in each psum group: they depend
        # only on Q16, so PE can run them before ctxT lands (and stay at
        # full p-state); the ctx matmuls close the group. Returns the
        # matmul closures so callers can splice them between score tiles.
        QT16 = QT16s[b]
        if fo == 0:
            st[b]["outT"] = p_out.tile([128, 4, L], F16, tag="outT",
                                       name=f"outT_{b}")
        pool = psC if (b == 1 and fo == 3) else psS
        ps = pool.tile([128, 1024], F32, tag="C" if pool is psC else "S",
                       name=f"o{b}{fo}")
        st[(b, fo, "ops")] = ps
        mms = []
        for nh in range(2):
            sl = slice(nh * 512, (nh + 1) * 512)
            for c in range(4):
                mms.append(lambda sl=sl, c=c: nc.tensor.matmul(
                    ps[:, sl], WoB_sb[:, c, fo * 128:(fo + 1) * 128],
                    QT16[:, c, sl], start=(c == 0), stop=False))
        return mms

    def outproj_ctx_mms(b, fo):
        ps = st[(b, fo, "ops")]
        mms = []
        for nh in range(2):
            sl = slice(nh * 512, (nh + 1) * 512)
            # ctxT [128, 8, 128] u16; fp8 dims (p, (qt cp), (qq i)) with
            # value(hd=256cp+2p+i, q=512nh+128qt+qq); DR rhs [p, i, qt, qq]
            ctx8r = st[b, "ctxT", nh].bitcast(FP8).rearrange(
                "p (a c) (q i) -> p c i a q", c=2, i=2)
            for cp in range(2):
                mms.append(lambda sl=sl, cp=cp, ctx8r=ctx8r:
                           nc.tensor.matmul(
                    ps[:, sl], WoT_sb[:, cp, :, fo * 128:(fo + 1) * 128],
                    ctx8r[:, cp, :, :, :],
                    start=False, stop=(cp == 1), perf_mode=DR))
        return mms

    def outproj_fin(b, fo):
        outT = st[b]["outT"]
        ps = st[(b, fo, "ops")]
        nc.scalar.activation(
            out=outT[:, fo, :], in_=ps, func=Tanh,
            bias=bo_sb[:, fo:fo + 1], scale=1.0 / (WO_SCALE * WO_SCALE))
        nc.sync.dma_start(out=dOut[b, fo * 128:(fo + 1) * 128, :],
                          in_=outT[:, fo, :])

    def outproj_ctx(b, fo):
        for m in outproj_ctx_mms(b, fo):
            m()
        outproj_fin(b, fo)

    def outproj(b, fo):
        for m in outproj_skip(b, fo):
            m()
        outproj_ctx(b, fo)

    def pe_filler(n, name, lhsT=None, rhs=None, pool=None):
        # redundant matmuls into a scratch psum tile: keeps the PE
        # p-state hot (ramp needs 3us of continuous busy for full clock)
        pool = pool or psS
        scratch = pool.tile([128, 512], F32,
                            tag="C" if pool is psC else "S", name=name)
        for i in range(n):
            nc.tensor.matmul(
                scratch,
                lhsT if lhsT is not None else WoB_sb[:, i % 4, 0:128],
                rhs if rhs is not None else QT16s[1][:, i % 4, 0:512],
                start=True, stop=True)

    # ---- software-pipelined emission ----------------------------------
    for b in range(BPC):
        st.setdefault(b, {})
    pe_filler(WARM_N, "warmup", lhsT=warm[:, 0:128], rhs=warm)
    proj_qk(0, (0, 1))
    st[0]["ctx_n"] = p_cn.tile([128, 8, H, D], FP8, tag="cn", name="cn_0")
    proj_v(0)
    scores(0, 0)
    proj_qk(0, (2, 3))
    for h in range(1, H):
        head_ctx(0, h - 1)
        scores(0, h)
        if h == 2:
            proj_qk(1, (0, 1))
            st[1]["ctx_n"] = p_cn.tile([128, 8, H, D], FP8, tag="cn",
                                       name="cn_1")
        if h == 3:
            proj_qk(1, (2, 3))
        if h == 4:
            proj_v(1)
    head_ctx(0, H - 1)
    ctx_transpose(0)
    scores(1, 0)
    for h in range(1, H):
        head_ctx(1, h - 1)
        if 1 <= h <= 4:
            fo = h - 1
            sp = outproj_skip(0, fo) + outproj_ctx_mms(0, fo)
            scores(1, h, splice=sp)
            outproj_fin(0, fo)
        else:
            scores(1, h)
    head_ctx(1, H - 1)
    ctx_transpose(1)
    for fo in range(3):
        for m in outproj_skip(1, fo):
            m()
    pe_filler(TAIL_N, "fill_tail", pool=psC)
    outproj_ctx(1, 0)
    outproj_ctx(1, 1)
    sk3 = outproj_skip(1, 3)
    for m in sk3:
        m()
    outproj_ctx(1, 2)
    outproj_ctx(1, 3)

    ctx.close()


def _host_prep(Wq, bq, Wk, Wv, Wo, bv, bo):
    """Host-side weight layout + scaling. Returns the wall blob."""
    f32 = np.float32
    # sigma column permutation for q/k projections: chunk ch=(g,pl),
    # partition 32j+delta -> head 4g+j, d = 32 pl + delta
    colmap = np.empty(F, dtype=np.int64)
    for ch in range(4):
        g, pl = ch // 2, ch % 2
        for j in range(4):
            for dlt in range(32):
                m = ch * 128 + 32 * j + dlt
                colmap[m] = 64 * (4 * g + j) + 32 * pl + dlt

    def pack_w(Wmat, cmap=None, scale=W_SCALE):
        # [128, u(2), i(2), 512] with rows f = 128*(2u+i) + p
        Wp = Wmat if cmap is None else Wmat[:, cmap]
        out = np.empty((128, 2, 2, F), dtype=E4M3)
        for u in range(2):
            for i in range(2):
                base = 128 * (2 * u + i)
                out[:, u, i, :] = (scale * Wp[base:base + 128, :]).astype(E4M3)
        return out.reshape(128, 2 * 2 * F)

    WoT = Wo[:F, :].astype(f32)
    WoB = Wo[F:, :].astype(f32)
    bo_eff = bo.astype(f32) + bv.astype(f32) @ WoT  # bv folds through WoT

    # WoT8: [128, cp(2), i(2), 512] rows hd = 256*cp + 2*p + i
    wot8 = np.empty((128, 2, 2, F), dtype=E4M3)
    for cp in range(2):
        for i in range(2):
            rows = 256 * cp + 2 * np.arange(128) + i
            wot8[:, cp, i, :] = (WO_SCALE * WoT[rows, :]).astype(E4M3)
    # WoB16: [128, c(4), 512] rows f = 128c + p
    wob16 = np.stack([WoB[128 * c:128 * (c + 1), :] for c in range(4)], axis=1)
    wob16 = (wob16 * WO_SCALE).astype(np.float16)

    bq_p = np.ascontiguousarray(
        (ALPHA * bq.astype(f32))[colmap].reshape(4, 128).T).astype(f32)
    bo_p = np.ascontiguousarray(bo_eff.reshape(4, 128).T).astype(f32)

    blob = np.concatenate([
        np.ascontiguousarray(bq_p).view(np.uint8),
        np.ascontiguousarray(bo_p).view(np.uint8),
        pack_w(Wq.astype(f32), colmap).view(np.uint8),
        pack_w(Wk.astype(f32), colmap).view(np.uint8),
        pack_w(Wv.astype(f32)).view(np.uint8),
        wot8.reshape(128, 2 * 2 * F).view(np.uint8),
        np.ascontiguousarray(wob16.reshape(128, 4 * F)).view(np.uint8),
    ], axis=1)
    assert blob.shape == (128, 12320), blob.shape
    return {"wall": blob}


def kernel(Q, K, V, Wq, bq, Wk, bk, Wv, bv, Wo, bo):
    if "nc" not in _CACHE:
        _CACHE["nc"] = _build_program()
    nc = _CACHE["nc"]

    f32 = np.float32
    Q, K, V = (np.asarray(x, dtype=f32) for x in (Q, K, V))

    in_common = _host_prep(np.asarray(Wq, f32), np.asarray(bq, f32),
                           np.asarray(Wk, f32), np.asarray(Wv, f32),
                           np.asarray(Wo, f32), np.asarray(bv, f32),
                           np.asarray(bo, f32))

    def packT8(X):  # (B, L, F) f32 -> (B, F, L) e4m3 bytes
        X8 = np.ascontiguousarray(X.astype(E4M3).transpose(0, 2, 1))
        return X8.view(np.uint8)

    Q16 = np.ascontiguousarray(
        (WO_SCALE * Q).astype(np.float16).transpose(0, 2, 1))
    QKV = np.stack([packT8(Q), packT8(K), packT8(V)], axis=2)

    in_maps = []
    for c in range(NCORES):
        s = slice(c * BPC, (c + 1) * BPC)
        in_maps.append({"qkv": QKV[s], "q16": Q16[s], **in_common})

    _CACHE["in_maps"] = in_maps
    res = bass_utils.run_bass_kernel_spmd(nc, in_maps,
                                          core_ids=list(range(NCORES)))
    # outT is (BPC, F, L) fp16 -> (B, L, F) fp32
    out = np.concatenate(
        [r["outT"].transpose(0, 2, 1) for r in res.results], axis=0)
    return np.ascontiguousarray(out).astype(f32)


def _last_in_maps():
    return _CACHE["in_maps"]


# revision 9
# speedup vs baseline: 1.0017x; 1.0017x over previous
"""MultiHeadAttention TRN2 Bass kernel, v2 (balanced ACT/DVE softmax).

Problem: B=16, L=1024, F=512, H=8 heads, D=64.
  q = Q@Wq+bq; k = K@Wk+bk(=0-folded); v = V@Wv+bv
  S = q k^T / sqrt(D); P = softmax(S); ctx = P v
  out = tanh(concat([ctx, Q]) @ Wo + bo)

Sharding: data-parallel over batch, 2 batches per core, 8 cores.

v2 design (cost-model driven; bottleneck = PSUM-evacuation on ACT+DVE):
  - All inputs host-pre-TRANSPOSED so every load is a plain DMA (no XBAR),
    roughly halving DMA-device occupancy vs transposed loads.
  - fp8e4m3 DoubleRow matmuls for proj / scores / ctx / out-proj(ctx half);
    fp16 skip path (precision-critical: concat-Q @ WoB).
  - Scores pre-scaled by A16=2^10/ln2 (folded into q/k evac scales).
    All P is stored e5m2; exp is split between ACT and DVE per kt tile
    (per-head ratios tuned by a greedy search over the cost model):
      * ACT tiles: exact exp activation -> e5m2.
      * DVE tiles: Schraudolph exp via ONE tensor_scalar:
        uint8(round(S~/256 + C_ADD)) bitcast as e5m2 IS exp(S-SHIFT);
        uint8 conversion saturates at 0 => tiny P for very negative S,
        and bits<=127 for S<=13, so no overflow either.
    ctx matmuls consume kt pairs as e5m2 DoubleRow (0.5 cyc/col).
  - ctx layout B: psum [128q, 8qt, D+1] with an appended ones column on v
    giving the softmax denominator Z for free; normalize = one reciprocal
    + one scalar_tensor_tensor per head (DVE), writing fp8 ctx_n.
  - ctx transpose for the output projection is done by the DMA XBAR
    (SBUF->SBUF u16-pair transposes), costing zero ACT/DVE/PE time.
  - out-proj psum [128f, L] per fo-chunk: 2 DR (ctx) + 4 fp16 (skip)
    matmuls, tanh evac on ACT with per-partition bo bias; outputs stored
    transposed (host undoes for free).
"""

import numpy as np
import ml_dtypes

import concourse.bass as bass
import concourse.tile as tile
from concourse import bacc, mybir
from concourse import bass_utils

B, L, F, H, D = 16, 1024, 512, 8, 64
NCORES = 8
BPC = B // NCORES
F32 = mybir.dt.float32
F16 = mybir.dt.float16
FP8 = mybir.dt.float8e4
FP8E5 = mybir.dt.float8e5
U16 = mybir.dt.uint16
U8 = mybir.dt.uint8

A16 = 1024.0 / np.log(2.0)        # 1477.32; S~ = A16 * S_true in psum
ALPHA = float(np.sqrt(A16 / 8.0))  # q/k evac scale so S~ = A16*S
SHIFT = 3.0                        # P = exp(S - SHIFT); S<=8.2 so P<=180<240
C_ADD = 60.0 - 4.0 * SHIFT / np.log(2.0) - 0.3  # e5m2 Schraudolph bias
W_SCALE = 32.0
CTX_SCALE = 16.0
WO_SCALE = 16.0

# exp engine per (batch, head, kt): A=ACT exact-exp, D=DVE schraudolph;
# all P is e5m2 so the engine choice is free per tile; alternate for
# pipelining. Per-head ACT counts found by greedy search over the
# TimelineSim makespan (correctness is invariant to this choice).
_PATS = {
    3: ("A", "D", "D", "A", "D", "D", "A", "D"),
    4: ("A", "D", "A", "D", "A", "D", "D", "A"),
    5: ("A", "D", "A", "D", "A", "D", "A", "A"),
    6: ("A", "A", "D", "A", "D", "A", "A", "A"),
}
_ACNT = {(0, 0): 4, (0, 1): 4, (0, 2): 5, (0, 3): 5, (0, 4): 5, (0, 5): 4,
         (0, 6): 5, (0, 7): 4, (1, 0): 5, (1, 1): 5, (1, 2): 6, (1, 3): 4,
         (1, 4): 5, (1, 5): 4, (1, 6): 5, (1, 7): 4}
TILE_ENG = {}
for _b in range(BPC):
    for _h in range(H):
        for _kt in range(8):
            TILE_ENG[(_b, _h, _kt)] = _PATS[_ACNT[(_b, _h)]][_kt]
# per-tile overrides from a randomized local search over the simulated
# makespan (correctness is invariant to the engine assignment)
TILE_ENG.update({
    (0, 0, 2): "D", (0, 0, 5): "A", (0, 2, 5): "A", (0, 2, 6): "D",
    (0, 3, 5): "A", (0, 3, 6): "D", (1, 1, 0): "D", (1, 1, 7): "D",
    (1, 2, 6): "D", (1, 3, 5): "A", (1, 3, 7): "D", (1, 4, 7): "D",
    (1, 5, 3): "A", (1, 7, 5): "A",
})

# evac engines per chunk parity / v-pair parity, filler counts (searched)
EVACQ = ("D", "A", "D", "A")   # per ch
EVACK = ("D", "A", "D", "A")
EVACV = ("D", "A", "D", "A")   # per kt-pair j
WARM_N = 8
TAIL_N = 6

_CACHE = {}
E4M3 = ml_dtypes.float8_e4m3


def _build_program():
    nc = bacc.Bacc("TRN2", target_bir_lowering=False)

    dIn = nc.dram_tensor("qkv", (BPC, F, 3, L), U8, kind="ExternalInput")
    dQ16 = nc.dram_tensor("q16", (BPC, F, L), F16, kind="ExternalInput")
    dWall = nc.dram_tensor("wall", (128, 12320), U8, kind="ExternalInput")
    dOut = nc.dram_tensor("outT", (BPC, F, L), F16, kind="ExternalOutput")

    with tile.TileContext(nc) as tc:
        _kernel(tc, dIn, dQ16, dWall, dOut)

    nc.compile()
    return nc


def _kernel(tc, dIn, dQ16, dWall, dOut):
    nc = tc.nc
    Exp = mybir.ActivationFunctionType.Exp
    Tanh = mybir.ActivationFunctionType.Tanh
    Ident = mybir.ActivationFunctionType.Identity
    Copy = mybir.ActivationFunctionType.Copy
    Mult = mybir.AluOpType.mult
    Add = mybir.AluOpType.add
    DR = mybir.MatmulPerfMode.DoubleRow

    from contextlib import ExitStack
    ctx = ExitStack()
    consts = ctx.enter_context(tc.tile_pool(name="consts", bufs=1))
    p_in = ctx.enter_context(tc.tile_pool(name="p_in", bufs=2))
    p_qk = ctx.enter_context(tc.tile_pool(name="p_qk", bufs=2))
    p_va = ctx.enter_context(tc.tile_pool(name="p_va", bufs=2))
    p_exp = ctx.enter_context(tc.tile_pool(name="p_exp", bufs=3))
    p_cn = ctx.enter_context(tc.tile_pool(name="p_cn", bufs=2))
    p_out = ctx.enter_context(tc.tile_pool(name="p_out", bufs=2))
    p_rc = ctx.enter_context(tc.tile_pool(name="p_rc", bufs=2))
    psS = ctx.enter_context(tc.tile_pool(name="psS", bufs=3, space="PSUM"))
    psC = ctx.enter_context(tc.tile_pool(name="psC", bufs=1, space="PSUM"))

    # ------------------------- loads -----------------------------------
    INs, QT16s = [], []
    for b in range(BPC):
        INs.append(p_in.tile([128, 4, 3, L], U8, tag="inall", name=f"in_{b}"))
        QT16s.append(p_in.tile([128, 4, L], F16, tag="qt16", name=f"qt16_{b}"))

    wall = consts.tile([128, 12320], U8, tag="wall")

    def load_t(b, t):
        nc.sync.dma_start(
            out=INs[b][:, :, t, :],
            in_=dIn[b, :, t, :].rearrange("(c p) l -> p c l", c=4))

    def load_q16(b):
        nc.sync.dma_start(
            out=QT16s[b],
            in_=dQ16[b].rearrange("(c p) l -> p c l", c=4))

    def load_qk(b):
        nc.sync.dma_start(
            out=INs[b][:, :, 0:2, :],
            in_=dIn[b, :, 0:2, :].rearrange("(c p) t l -> p c t l", c=4))

    # biases+Wq+Wk first, then weights in consumption order; q16 last
    nc.sync.dma_start(out=wall[:, 0:4128], in_=dWall[:, 0:4128])
    load_t(0, 0)
    load_t(0, 1)
    nc.sync.dma_start(out=wall[:, 4128:8224], in_=dWall[:, 4128:8224])
    load_t(0, 2)
    nc.sync.dma_start(out=wall[:, 8224:12320], in_=dWall[:, 8224:12320])
    load_qk(1)
    load_t(1, 2)
    load_q16(0)
    load_q16(1)

    bq_sb = wall[:, 0:16].bitcast(F32)
    bo_sb = wall[:, 16:32].bitcast(F32)
    Wq_sb = wall[:, 32:2080].bitcast(FP8).rearrange(
        "p (u i f) -> p u i f", u=2, i=2)
    Wk_sb = wall[:, 2080:4128].bitcast(FP8).rearrange(
        "p (u i f) -> p u i f", u=2, i=2)
    Wv_sb = wall[:, 4128:6176].bitcast(FP8).rearrange(
        "p (u i f) -> p u i f", u=2, i=2)
    WoT_sb = wall[:, 6176:8224].bitcast(FP8).rearrange(
        "p (c i f) -> p c i f", c=2, i=2)
    WoB_sb = wall[:, 8224:12320].bitcast(F16).rearrange(
        "p (c f) -> p c f", c=4)
    nshift_sb = consts.tile([128, 1], F32, tag="nshift")
    nc.gpsimd.memset(nshift_sb, -SHIFT)
    warm = consts.tile([128, 512], F16, tag="warm")
    nc.gpsimd.memset(warm, 0.5)
    # dummy exp forces the ACT table load at t~0 instead of lazily at
    # the first real activation
    actprime = consts.tile([128, 1], F32, tag="actprime")
    nc.scalar.activation(out=actprime, in_=nshift_sb, func=Exp,
                         scale=1.0, bias=nshift_sb[:, 0:1])

    st = {}

    # ------------------------- projections -----------------------------
    def proj_qk(b, chs):
        ia = INs[b].bitcast(FP8)
        Q8, K8 = ia[:, :, 0, :], ia[:, :, 1, :]
        if chs[0] == 0:
            st[b]["qs8"] = p_qk.tile([128, 2, 2, L], FP8, tag="qs8",
                                     name=f"qs8_{b}")
            st[b]["ks8"] = p_qk.tile([128, 2, 2, L], FP8, tag="ks8",
                                     name=f"ks8_{b}")
        qs8, ks8 = st[b]["qs8"], st[b]["ks8"]
        for ch in chs:
            ps = psS.tile([128, 1024], F32, tag="S", name=f"qp{b}{ch}")
            for nh in range(2):
                sl = slice(nh * 512, (nh + 1) * 512)
                for u in range(2):
                    nc.tensor.matmul(
                        ps[:, sl], Wq_sb[:, u, :, ch * 128:(ch + 1) * 128],
                        Q8[:, 2 * u:2 * u + 2, sl],
                        start=(u == 0), stop=(u == 1), perf_mode=DR)
            if EVACQ[ch] == "A":
                nc.scalar.activation(
                    out=qs8[:, ch // 2, ch % 2, :], in_=ps, func=Ident,
                    bias=bq_sb[:, ch:ch + 1], scale=ALPHA / W_SCALE)
            else:
                nc.vector.tensor_scalar(
                    out=qs8[:, ch // 2, ch % 2, :], in0=ps,
                    scalar1=ALPHA / W_SCALE, scalar2=bq_sb[:, ch:ch + 1],
                    op0=Mult, op1=Add)
            ps = psS.tile([128, 1024], F32, tag="S", name=f"kp{b}{ch}")
            for nh in range(2):
                sl = slice(nh * 512, (nh + 1) * 512)
                for u in range(2):
                    nc.tensor.matmul(
                        ps[:, sl], Wk_sb[:, u, :, ch * 128:(ch + 1) * 128],
                        K8[:, 2 * u:2 * u + 2, sl],
                        start=(u == 0), stop=(u == 1), perf_mode=DR)
            if EVACK[ch] == "D":
                nc.vector.tensor_scalar(
                    out=ks8[:, ch // 2, ch % 2, :], in0=ps,
                    scalar1=ALPHA / W_SCALE, scalar2=None, op0=Mult)
            else:
                nc.scalar.activation(
                    out=ks8[:, ch // 2, ch % 2, :], in_=ps, func=Copy,
                    scale=ALPHA / W_SCALE)

    def proj_v(b):
        ia = INs[b].bitcast(FP8)
        V8 = ia[:, :, 2, :]
        vaug8 = p_va.tile([128, 8, H, D + 1], FP8, tag="v8", name=f"v8_{b}")
        st[b]["vaug8"] = vaug8
        nc.gpsimd.memset(vaug8[:, :, :, D:D + 1], 1.0)
        for j in range(4):
            ps = psS.tile([128, 1024], F32, tag="S", name=f"vp{b}{j}")
            psv = ps.rearrange("p (s n) -> p s n", s=2)
            for s in range(2):
                kt = 2 * j + s
                for u in range(2):
                    nc.tensor.matmul(
                        psv[:, s, :],
                        V8[:, 2 * u:2 * u + 2, kt * 128:(kt + 1) * 128],
                        Wv_sb[:, u, :, :],
                        start=(u == 0), stop=(u == 1), perf_mode=DR)
            if EVACV[j] == "D":
                nc.vector.tensor_scalar(
                    out=vaug8[:, 2 * j:2 * j + 2, :, 0:D],
                    in0=psv.rearrange("p s (h d) -> p s h d", h=H),
                    scalar1=1.0 / W_SCALE, scalar2=None, op0=Mult)
            else:
                nc.scalar.activation(
                    out=vaug8[:, 2 * j:2 * j + 2, :, 0:D],
                    in_=psv.rearrange("p s (h d) -> p s h d", h=H),
                    func=Copy, scale=1.0 / W_SCALE)

    # ------------------------- attention -------------------------------
    def scores(b, h, splice=()):
        # `splice`: PE-matmul closures interleaved between kt tiles so
        # bulky side work never blocks the scores pipeline refill
        splice = list(splice)
        qs8, ks8 = st[b]["qs8"], st[b]["ks8"]
        pb, gg = 32 * (h % 4), h // 4
        e8 = p_exp.tile([128, 8, L], FP8, tag="e8", name=f"e8_{b}_{h}")
        e8u = e8.bitcast(U8)
        st[(b, h)] = e8
        for kt in range(8):
            for _ in range(2):
                if splice:
                    splice.pop(0)()
            ps = psS.tile([128, 1024], F32, tag="S", name=f"s{b}{h}{kt}")
            for nh in range(2):
                sl = slice(nh * 512, (nh + 1) * 512)
                nc.tensor.matmul(
                    ps[:, sl], ks8[pb:pb + 32, gg, :, kt * 128:(kt + 1) * 128],
                    qs8[pb:pb + 32, gg, :, sl],
                    start=True, stop=True, perf_mode=DR,
                    tile_position=(pb, 0))
            if TILE_ENG[(b, h, kt)] == "A":
                nc.scalar.activation(
                    out=e8.bitcast(FP8E5)[:, kt, :], in_=ps, func=Exp,
                    scale=1.0 / A16, bias=nshift_sb[:, 0:1])
            else:
                nc.vector.tensor_scalar(
                    out=e8u[:, kt, :], in0=ps, scalar1=1.0 / 256.0,
                    scalar2=C_ADD, op0=Mult, op1=Add)

    def head_ctx(b, h):
        e8 = st[(b, h)]
        vaug8, ctx_n = st[b]["vaug8"], st[b]["ctx_n"]
        e85 = e8.bitcast(FP8E5)
        # single 2-bank cps tile per head; qt stride padded to 128 floats
        # (512B) so every accumulation group stays within one psum bank
        cps = psC.tile([128, 8, 128], F32, tag="C", name=f"c{b}{h}")
        for qt in range(8):
            for j in range(4):
                nc.tensor.matmul(
                    cps[:, qt, 0:D + 1],
                    e85[:, 2 * j:2 * j + 2, qt * 128:(qt + 1) * 128],
                    vaug8[:, 2 * j:2 * j + 2, h, :],
                    start=(j == 0), stop=(j == 3),
                    perf_mode=DR)
        rc = p_rc.tile([128, 8], F32, tag="rc", name=f"rc{b}{h}")
        nc.vector.reciprocal(out=rc, in_=cps[:, :, D])
        rcb = rc.unsqueeze(2).broadcast_to([128, 8, D])
        nc.vector.scalar_tensor_tensor(
            out=ctx_n[:, :, h, :],
            in0=cps[:, :, 0:D], scalar=CTX_SCALE, in1=rcb,
            op0=Mult, op1=Mult)

    def ctx_transpose(b):
        # two XBAR transposes (one per q-half) into SEPARATE tiles so the
        # tile framework doesn't serialize the writers:
        # out[p, g=2qt+cp, q] = cn16[q, 128*g + p]  (probe-verified)
        cn16 = st[b]["ctx_n"].bitcast(U16).rearrange(
            "p a h d -> p (a h d)")            # [128, 2048]
        for nh in range(2):
            ctxT = p_cn.tile([128, 8, 128], U16, tag=f"ctxT{nh}",
                             name=f"ctxT_{b}_{nh}")
            st[b, "ctxT", nh] = ctxT
            nc.sync.dma_start_transpose(
                out=ctxT, in_=cn16[:, nh * 1024:(nh + 1) * 1024])

    def outproj_skip(b, fo):
        # skip-path (Q16) matmuls FIRST in each psum group: they depend
        # only on Q16, so PE can run them before ctxT lands (and stay at
        # full p-state); the ctx matmuls close the group. Returns the
        # matmul closures so callers can splice them between score tiles.
        QT16 = QT16s[b]
        if fo == 0:
            st[b]["outT"] = p_out.tile([128, 4, L], F16, tag="outT",
                                       name=f"outT_{b}")
        pool = psC if (b == 1 and fo == 3) else psS
        ps = pool.tile([128, 1024], F32, tag="C" if pool is psC else "S",
                       name=f"o{b}{fo}")
        st[(b, fo, "ops")] = ps
        mms = []
        for nh in range(2):
            sl = slice(nh * 512, (nh + 1) * 512)
            for c in range(4):
                mms.append(lambda sl=sl, c=c: nc.tensor.matmul(
                    ps[:, sl], WoB_sb[:, c, fo * 128:(fo + 1) * 128],
                    QT16[:, c, sl], start=(c == 0), stop=False))
        return mms

    def outproj_ctx_mms(b, fo):
        ps = st[(b, fo, "ops")]
        mms = []
        for nh in range(2):
            sl = slice(nh * 512, (nh + 1) * 512)
            # ctxT [128, 8, 128] u16; fp8 dims (p, (qt cp), (qq i)) with
            # value(hd=256cp+2p+i, q=512nh+128qt+qq); DR rhs [p, i, qt, qq]
            ctx8r = st[b, "ctxT", nh].bitcast(FP8).rearrange(
                "p (a c) (q i) -> p c i a q", c=2, i=2)
            for cp in range(2):
                mms.append(lambda sl=sl, cp=cp, ctx8r=ctx8r:
                           nc.tensor.matmul(
                    ps[:, sl], WoT_sb[:, cp, :, fo * 128:(fo + 1) * 128],
                    ctx8r[:, cp, :, :, :],
                    start=False, stop=(cp == 1), perf_mode=DR))
        return mms

    def outproj_fin(b, fo):
        outT = st[b]["outT"]
        ps = st[(b, fo, "ops")]
        nc.scalar.activation(
            out=outT[:, fo, :], in_=ps, func=Tanh,
            bias=bo_sb[:, fo:fo + 1], scale=1.0 / (WO_SCALE * WO_SCALE))
        nc.sync.dma_start(out=dOut[b, fo * 128:(fo + 1) * 128, :],
                          in_=outT[:, fo, :])

    def outproj_ctx(b, fo):
        for m in outproj_ctx_mms(b, fo):
            m()
        outproj_fin(b, fo)

    def outproj(b, fo):
        for m in outproj_skip(b, fo):
            m()
        outproj_ctx(b, fo)

    def pe_filler(n, name, lhsT=None, rhs=None, pool=None):
        # redundant matmuls into a scratch psum tile: keeps the PE
        # p-state hot (ramp needs 3us of continuous busy for full clock)
        pool = pool or psS
        scratch = pool.tile([128, 512], F32,
                            tag="C" if pool is psC else "S", name=name)
        for i in range(n):
            nc.tensor.matmul(
                scratch,
                lhsT if lhsT is not None else WoB_sb[:, i % 4, 0:128],
                rhs if rhs is not None else QT16s[1][:, i % 4, 0:512],
                start=True, stop=True)

    # ---- software-pipelined emission ----------------------------------
    for b in range(BPC):
        st.setdefault(b, {})
    pe_filler(WARM_N, "warmup", lhsT=warm[:, 0:128], rhs=warm)
    proj_qk(0, (0, 1))
    st[0]["ctx_n"] = p_cn.tile([128, 8, H, D], FP8, tag="cn", name="cn_0")
    proj_v(0)
    scores(0, 0)
    proj_qk(0, (2, 3))
    for h in range(1, H):
        head_ctx(0, h - 1)
        scores(0, h)
        if h == 2:
            proj_qk(1, (0, 1))
            st[1]["ctx_n"] = p_cn.tile([128, 8, H, D], FP8, tag="cn",
                                       name="cn_1")
        if h == 3:
            proj_qk(1, (2, 3))
        if h == 4:
            proj_v(1)
    head_ctx(0, H - 1)
    ctx_transpose(0)
    scores(1, 0)
    for h in range(1, H):
        head_ctx(1, h - 1)
        if 1 <= h <= 4:
            fo = h - 1
            sp = outproj_skip(0, fo) + outproj_ctx_mms(0, fo)
            scores(1, h, splice=sp)
            outproj_fin(0, fo)
        else:
            scores(1, h)
    head_ctx(1, H - 1)
    ctx_transpose(1)
    for fo in range(3):
        for m in outproj_skip(1, fo):
            m()
    pe_filler(TAIL_N, "fill_tail", pool=psC)
    outproj_ctx(1, 0)
    outproj_ctx(1, 1)
    sk3 = outproj_skip(1, 3)
    for m in sk3:
        m()
    outproj_ctx(1, 2)
    outproj_ctx(1, 3)

    ctx.close()


def _host_prep(Wq, bq, Wk, Wv, Wo, bv, bo):
    """Host-side weight layout + scaling. Returns the wall blob."""
    f32 = np.float32
    # sigma column permutation for q/k projections: chunk ch=(g,pl),
    # partition 32j+delta -> head 4g+j, d = 32 pl + delta
    colmap = np.empty(F, dtype=np.int64)
    for ch in range(4):
        g, pl = ch // 2, ch % 2
        for j in range(4):
            for dlt in range(32):
                m = ch * 128 + 32 * j + dlt
                colmap[m] = 64 * (4 * g + j) + 32 * pl + dlt

    def pack_w(Wmat, cmap=None, scale=W_SCALE):
        # [128, u(2), i(2), 512] with rows f = 128*(2u+i) + p
        Wp = Wmat if cmap is None else Wmat[:, cmap]
        out = np.empty((128, 2, 2, F), dtype=E4M3)
        for u in range(2):
            for i in range(2):
                base = 128 * (2 * u + i)
                out[:, u, i, :] = (scale * Wp[base:base + 128, :]).astype(E4M3)
        return out.reshape(128, 2 * 2 * F)

    WoT = Wo[:F, :].astype(f32)
    WoB = Wo[F:, :].astype(f32)
    bo_eff = bo.astype(f32) + bv.astype(f32) @ WoT  # bv folds through WoT

    # WoT8: [128, cp(2), i(2), 512] rows hd = 256*cp + 2*p + i
    wot8 = np.empty((128, 2, 2, F), dtype=E4M3)
    for cp in range(2):
        for i in range(2):
            rows = 256 * cp + 2 * np.arange(128) + i
            wot8[:, cp, i, :] = (WO_SCALE * WoT[rows, :]).astype(E4M3)
    # WoB16: [128, c(4), 512] rows f = 128c + p
    wob16 = np.stack([WoB[128 * c:128 * (c + 1), :] for c in range(4)], axis=1)
    wob16 = (wob16 * WO_SCALE).astype(np.float16)

    bq_p = np.ascontiguousarray(
        (ALPHA * bq.astype(f32))[colmap].reshape(4, 128).T).astype(f32)
    bo_p = np.ascontiguousarray(bo_eff.reshape(4, 128).T).astype(f32)

    blob = np.concatenate([
        np.ascontiguousarray(bq_p).view(np.uint8),
        np.ascontiguousarray(bo_p).view(np.uint8),
        pack_w(Wq.astype(f32), colmap).view(np.uint8),
        pack_w(Wk.astype(f32), colmap).view(np.uint8),
        pack_w(Wv.astype(f32)).view(np.uint8),
        wot8.reshape(128, 2 * 2 * F).view(np.uint8),
        np.ascontiguousarray(wob16.reshape(128, 4 * F)).view(np.uint8),
    ], axis=1)
    assert blob.shape == (128, 12320), blob.shape
    return {"wall": blob}


def kernel(Q, K, V, Wq, bq, Wk, bk, Wv, bv, Wo, bo):
    if "nc" not in _CACHE:
        _CACHE["nc"] = _build_program()
    nc = _CACHE["nc"]

    f32 = np.float32
    Q, K, V = (np.asarray(x, dtype=f32) for x in (Q, K, V))

    in_common = _host_prep(np.asarray(Wq, f32), np.asarray(bq, f32),
                           np.asarray(Wk, f32), np.asarray(Wv, f32),
                           np.asarray(Wo, f32), np.asarray(bv, f32),
                           np.asarray(bo, f32))

    def packT8(X):  # (B, L, F) f32 -> (B, F, L) e4m3 bytes
        X8 = np.ascontiguousarray(X.astype(E4M3).transpose(0, 2, 1))
        return X8.view(np.uint8)

    Q16 = np.ascontiguousarray(
        (WO_SCALE * Q).astype(np.float16).transpose(0, 2, 1))
    QKV = np.stack([packT8(Q), packT8(K), packT8(V)], axis=2)

    in_maps = []
    for c in range(NCORES):
        s = slice(c * BPC, (c + 1) * BPC)
        in_maps.append({"qkv": QKV[s], "q16": Q16[s], **in_common})

    _CACHE["in_maps"] = in_maps
    res = bass_utils.run_bass_kernel_spmd(nc, in_maps,
                                          core_ids=list(range(NCORES)))
    # outT is (BPC, F, L) fp16 -> (B, L, F) fp32
    out = np.concatenate(
        [r["outT"].transpose(0, 2, 1) for r in res.results], axis=0)
    return np.ascontiguousarray(out).astype(f32)


def _last_in_maps():
    return _CACHE["in_maps"]
